# revision 19
# baseline (speedup 1.0000x reference)
"""GNN NodeBlock kernel for 8x TRN2 NeuronCores.

Strategy: shard NODES (receivers) across the 8 cores; the host routes
each edge to the core owning its receiver, so aggregation is fully
local. On each core, nodes are bin-packed (LPT on degree) into 208
windows of 64 nodes whose edge tokens fit 5x128-token tiles; the
kernel builds one-hot routing matrices on-chip (iota==slot compare)
and computes each window's segment sum as a PSUM-accumulated fp32r
matmul. Mean + PE-transpose + fp32r matmul MLP (global_attr gather
folded into a host-built one-hot) produce the output rows, which the
host un-permutes.
"""

import heapq

import numpy as np
from contextlib import ExitStack

import concourse.bass as bass
import concourse.tile as tile
from concourse import bacc, mybir
from concourse.bass import AP
from concourse.bass_utils import run_bass_kernel_spmd

N_NODES = 100000
N_EDGES = 1000000
D = 64
NB = 64
LATENT = 32
OUT_DIM = 64

NCORES = 8
NPC = N_NODES // NCORES      # 12500 nodes per core
WIN = 64                     # nodes per window
NW = 208                     # windows per core
NSLOT = NW * WIN             # 13312 node slots (>= NPC)
NBLK = NSLOT // 128          # 104 output row-blocks
NSUP = NBLK // 4             # 26 supertiles of 512 nodes / 8 windows
TPW = 5                      # 128-token tiles per window
WTOK = TPW * 128             # 640 edge-token capacity per window
NT = NW * TPW                # 1040 token tiles per core
CAPT = NT * 128              # 133120 token slots per core
EW = D + 1                   # payload: 64 feats + 1.0 count flag
EWP = 66                     # padded to even (fp32r matmul dst restriction)
F32 = mybir.dt.float32
F32R = mybir.dt.float32r
EQ = mybir.AluOpType.is_equal
MUL = mybir.AluOpType.mult
Copy = mybir.ActivationFunctionType.Copy
Relu = mybir.ActivationFunctionType.Relu

_PROG = None


def _bcast(ap, dim, n):
    """Insert a zero-stride dim of size n at free-dim position dim."""
    layout = list(ap.ap)
    layout.insert(1 + dim, [0, n])
    return AP(ap.tensor, ap.offset, layout)


def _build_program(reps=1):
    nc = bacc.Bacc(None, target_bir_lowering=False, debug=True)

    edges_d = nc.dram_tensor("edges_tok", [128, NT, EWP], F32R, kind="ExternalInput")
    ridx_d = nc.dram_tensor("ridx", [128, NT], F32, kind="ExternalInput")
    nodeT_d = nc.dram_tensor("nodeT", [D, NSLOT], F32R, kind="ExternalInput")
    onehot_d = nc.dram_tensor("onehot", [NB, NSLOT], F32R, kind="ExternalInput")
    gaT_d = nc.dram_tensor("gaT", [D, NB], F32R, kind="ExternalInput")
    w1n_d = nc.dram_tensor("w1n", [D, LATENT], F32R, kind="ExternalInput")
    w1a_d = nc.dram_tensor("w1a", [D, LATENT], F32R, kind="ExternalInput")
    w1g_d = nc.dram_tensor("w1g", [D, LATENT], F32R, kind="ExternalInput")
    w2_d = nc.dram_tensor("w2", [LATENT, OUT_DIM], F32R, kind="ExternalInput")
    b1_d = nc.dram_tensor("b1c", [LATENT, 1], F32, kind="ExternalInput")
    b2_d = nc.dram_tensor("b2b", [128, OUT_DIM], F32, kind="ExternalInput")
    ident_d = nc.dram_tensor("ident", [128, 128], F32, kind="ExternalInput")
    iota_d = nc.dram_tensor("iota", [128, WIN], F32, kind="ExternalInput")
    out_d = nc.dram_tensor("out", [NSLOT, OUT_DIM], F32, kind="ExternalOutput")

    with tile.TileContext(nc) as tc:
     # body repeated `reps` times for delta-timing (overhead cancels)
     for _rep in range(reps):
      with ExitStack() as stk:
        persist = stk.enter_context(tc.tile_pool(name="persist", bufs=1))
        gaT = persist.tile([D, NB], F32R)
        w1n = persist.tile([D, LATENT], F32R)
        w1a = persist.tile([D, LATENT], F32R)
        w1g = persist.tile([D, LATENT], F32R)
        w2 = persist.tile([LATENT, OUT_DIM], F32R)
        b1c = persist.tile([LATENT, 1], F32)
        b2b = persist.tile([128, OUT_DIM], F32)
        ident = persist.tile([128, 128], F32)
        iota = persist.tile([128, WIN], F32)
        g_sb = persist.tile([NB, LATENT], F32R)
        ridx_sb = persist.tile([128, NT], F32)

        for sb, dr in ((gaT, gaT_d), (w1n, w1n_d), (w1a, w1a_d), (w1g, w1g_d),
                       (w2, w2_d), (b1c, b1_d), (b2b, b2_d), (ident, ident_d),
                       (iota, iota_d), (ridx_sb, ridx_d)):
            nc.sync.dma_start(sb[:], dr[:])

        # G = global_attr @ W1g  (per-batch hidden contribution)
        with tc.tile_pool(name="psg", bufs=1, space="PSUM") as psg:
            ps_g = psg.tile([NB, LATENT], F32)
            nc.tensor.matmul(ps_g[:], gaT[:], w1g[:], start=True, stop=True)
            nc.scalar.activation(g_sb[:], ps_g[:], Copy)

        ppool = stk.enter_context(tc.tile_pool(name="pt", bufs=2))
        ohpool = stk.enter_context(tc.tile_pool(name="ohb", bufs=2))
        npool = stk.enter_context(tc.tile_pool(name="ndT", bufs=2))
        gpool = stk.enter_context(tc.tile_pool(name="ghT", bufs=2))
        pkpool = stk.enter_context(tc.tile_pool(name="pk", bufs=2))
        scpool = stk.enter_context(tc.tile_pool(name="sc", bufs=2))
        aggp = stk.enter_context(tc.tile_pool(name="agg", bufs=2))
        hp = stk.enter_context(tc.tile_pool(name="hp", bufs=2))
        opool = stk.enter_context(tc.tile_pool(name="op", bufs=4))
        psa = stk.enter_context(tc.tile_pool(name="psa", bufs=2, space="PSUM"))
        pst = stk.enter_context(tc.tile_pool(name="pst", bufs=2, space="PSUM"))
        ps1p = stk.enter_context(tc.tile_pool(name="ps1", bufs=2, space="PSUM"))
        ps2p = stk.enter_context(tc.tile_pool(name="ps2", bufs=2, space="PSUM"))

        WT = 8 * TPW  # 40 token tiles per supertile
        for s in range(NSUP):
            nodeT = npool.tile([D, 512], F32R, name="ndT")
            ohg = gpool.tile([NB, 512], F32R, name="ghT")
            nc.sync.dma_start(nodeT[:], nodeT_d[:, 512 * s:512 * (s + 1)])
            nc.sync.dma_start(ohg[:], onehot_d[:, 512 * s:512 * (s + 1)])

            pt = ppool.tile([128, WT, EWP], F32R, name="pt")
            nc.sync.dma_start(pt[:], edges_d[:, WT * s:WT * (s + 1), :])

            # one-hot routing for all 40 tiles at once
            ohb = ohpool.tile([128, WT, WIN], F32R, name="ohb")
            nc.vector.tensor_tensor(
                ohb[:], _bcast(iota[:], 0, WT),
                _bcast(ridx_sb[:, WT * s:WT * (s + 1)], 1, WIN), op=EQ)

            packed = pkpool.tile([WIN, 8, EWP], F32, name="pk")
            for wi in range(8):
                ps_agg = psa.tile([WIN, EWP], F32, name="ps_agg")
                for j in range(TPW):
                    t = TPW * wi + j
                    nc.tensor.matmul(
                        ps_agg[:], ohb[:, t, :], pt[:, t, :],
                        start=(j == 0), stop=(j == TPW - 1))
                nc.scalar.activation(packed[:, wi, :], ps_agg[:], Copy)

            cnt = scpool.tile([WIN, 8], F32, name="cnt")
            nc.vector.tensor_scalar_max(cnt[:], packed[:, :, D], 1.0)
            recip = scpool.tile([WIN, 8], F32, name="recip")
            nc.vector.reciprocal(recip[:], cnt[:])
            scaled = scpool.tile([WIN, 8, D], F32, name="scaled")
            nc.vector.tensor_tensor(scaled[:], packed[:, :, 0:D],
                                    _bcast(recip[:], 1, D), op=MUL)

            aggT = aggp.tile([D, 512], F32R, name="aggT")
            for wi in range(8):
                ps_t = pst.tile([D, WIN], F32, name="ps_t")
                nc.tensor.transpose(ps_t[:], scaled[:, wi, :],
                                    ident[0:WIN, 0:WIN])
                nc.scalar.activation(aggT[:, WIN * wi:WIN * (wi + 1)],
                                     ps_t[:], Copy)

            ps1 = ps1p.tile([LATENT, 512], F32, name="ps1")
            nc.tensor.matmul(ps1[:], w1n[:], nodeT[:], start=True, stop=False)
            nc.tensor.matmul(ps1[:], w1a[:], aggT[:], start=False, stop=False)
            nc.tensor.matmul(ps1[:], g_sb[:], ohg[:], start=False, stop=True)
            h = hp.tile([LATENT, 512], F32R, name="h")
            nc.scalar.activation(h[:], ps1[:], Relu, bias=b1c[:])
            for q in range(4):
                j = 4 * s + q
                ps2 = ps2p.tile([128, OUT_DIM], F32, name="ps2")
                nc.tensor.matmul(ps2[:], h[:, 128 * q:128 * (q + 1)], w2[:],
                                 start=True, stop=True)
                ob = opool.tile([128, OUT_DIM], F32, name="ob")
                nc.vector.tensor_add(ob[:], ps2[:], b2b[:])
                nc.sync.dma_start(out_d[128 * j:128 * (j + 1), :], ob[:])

    nc.compile()
    return nc


def _pack_windows(deg):
    """LPT bin-packing: assign each node to a window, balancing edge
    load with caps of WIN nodes / WTOK edges per window."""
    win_of = np.empty(NPC, np.int32)
    slot_of = np.empty(NPC, np.int32)
    counts = np.zeros(NW, np.int32)
    loads = np.zeros(NW, np.int64)
    heap = [(0, w) for w in range(NW)]
    for n in np.argsort(-deg, kind="stable"):
        while True:
            load, w = heapq.heappop(heap)
            if counts[w] < WIN:
                break
        win_of[n] = w
        slot_of[n] = counts[w]
        counts[w] += 1
        loads[w] += deg[n]
        assert loads[w] <= WTOK, f"window {w} overflow: {loads[w]}"
        if counts[w] < WIN:
            heapq.heappush(heap, (int(loads[w]), w))
    return win_of, slot_of


def _prep_inputs(node_attr, edge_attr, global_attr, W1, b1, W2, b2,
                 receivers_idx, ng_index):
    node_attr = np.asarray(node_attr, np.float32)
    edge_attr = np.asarray(edge_attr, np.float32)
    global_attr = np.asarray(global_attr, np.float32)
    W1 = np.asarray(W1, np.float32)
    b1 = np.asarray(b1, np.float32)
    W2 = np.asarray(W2, np.float32)
    b2 = np.asarray(b2, np.float32)
    receivers_idx = np.asarray(receivers_idx, np.int64)
    ng_index = np.asarray(ng_index, np.int64)

    shared = {
        "gaT": np.ascontiguousarray(global_attr.T),
        "w1n": np.ascontiguousarray(W1[0:D]),
        "w1a": np.ascontiguousarray(W1[D:2 * D]),
        "w1g": np.ascontiguousarray(W1[2 * D:3 * D]),
        "w2": np.ascontiguousarray(W2),
        "b1c": np.ascontiguousarray(b1.reshape(LATENT, 1)),
        "b2b": np.ascontiguousarray(np.broadcast_to(b2, (128, OUT_DIM))),
        "ident": np.eye(128, dtype=np.float32),
        "iota": np.tile(np.arange(WIN, dtype=np.float32), (128, 1)),
    }

    order = np.argsort(receivers_idx, kind="stable")
    sorted_recv = receivers_idx[order]
    bounds = np.searchsorted(sorted_recv, np.arange(0, N_NODES + 1, NPC))

    in_maps = []
    perms = []
    for k in range(NCORES):
        sel = order[bounds[k]:bounds[k + 1]]
        lrecv = (sorted_recv[bounds[k]:bounds[k + 1]] - k * NPC).astype(np.int64)
        e = sel.size
        deg = np.bincount(lrecv, minlength=NPC)
        win_of, slot_of = _pack_windows(deg)

        ew = win_of[lrecv].astype(np.int64)
        ord2 = np.argsort(ew, kind="stable")
        sel2 = sel[ord2]
        lrecv2 = lrecv[ord2]
        ew2 = ew[ord2]
        starts = np.searchsorted(ew2, np.arange(NW))
        pos = np.arange(e) - starts[ew2]
        assert e == 0 or pos.max() < WTOK
        tokslot = ew2 * WTOK + pos

        tok = np.zeros((CAPT, EWP), np.float32)
        tok[tokslot, :D] = edge_attr[sel2]
        tok[tokslot, D] = 1.0
        edges_tok = np.ascontiguousarray(
            tok.reshape(NT, 128, EWP).transpose(1, 0, 2))
        rx = np.full(CAPT, -1.0, np.float32)
        rx[tokslot] = slot_of[lrecv2]
        ridx = np.ascontiguousarray(rx.reshape(NT, 128).T)

        perm = np.full(NSLOT, -1, np.int64)
        perm[win_of.astype(np.int64) * WIN + slot_of] = np.arange(NPC)
        valid = np.flatnonzero(perm >= 0)
        gids = k * NPC + perm[valid]
        nodeT = np.zeros((D, NSLOT), np.float32)
        nodeT[:, valid] = node_attr[gids].T
        oh = np.zeros((NB, NSLOT), np.float32)
        oh[ng_index[gids], valid] = 1.0

        m = {"edges_tok": edges_tok, "ridx": ridx, "nodeT": nodeT, "onehot": oh}
        m.update(shared)
        in_maps.append(m)
        perms.append(perm)
    return in_maps, perms


def _gather(outs, perms):
    full = np.zeros((N_NODES, OUT_DIM), np.float32)
    for k in range(NCORES):
        perm = perms[k]
        valid = np.flatnonzero(perm >= 0)
        full[k * NPC + perm[valid]] = np.asarray(outs[k])[valid]
    return full


def kernel(**inputs):
    global _PROG
    if _PROG is None:
        _PROG = _build_program()
    in_maps, perms = _prep_inputs(**inputs)
    res = run_bass_kernel_spmd(_PROG, in_maps, list(range(NCORES)), trace=False)
    return _gather([res.results[k]["out"] for k in range(NCORES)], perms)


# revision 28
# speedup vs baseline: 895.2116x; 895.2116x over previous
"""GNN NodeBlock kernel for 8x TRN2 NeuronCores.

Strategy: shard NODES (receivers) across the 8 cores; the host routes
each edge to the core owning its receiver, so aggregation is fully
local. On each core, nodes are bin-packed (LPT on degree) into 208
windows of 64 nodes whose edge tokens fit 5x128-token tiles; the
kernel builds one-hot routing matrices on-chip (iota==slot compare)
and computes each window's segment sum as a PSUM-accumulated fp32r
matmul. Mean + PE-transpose + fp32r matmul MLP (global_attr gather
folded into a host-built one-hot) produce the output rows, which the
host un-permutes.
"""

import heapq

import ml_dtypes
import numpy as np
from contextlib import ExitStack

import concourse.bass as bass
import concourse.tile as tile
from concourse import bacc, mybir
from concourse.bass import AP
from concourse.bass_utils import run_bass_kernel_spmd

N_NODES = 100000
N_EDGES = 1000000
D = 64
NB = 64
LATENT = 32
OUT_DIM = 64

NCORES = 8
NPC = N_NODES // NCORES      # 12500 nodes per core
WIN = 64                     # nodes per window
NW = 208                     # windows per core
NSLOT = NW * WIN             # 13312 node slots (>= NPC)
NBLK = NSLOT // 128          # 104 output row-blocks
NSUP = NBLK // 4             # 26 supertiles of 512 nodes / 8 windows
TPW = 5                      # 128-token tiles per window
WTOK = TPW * 128             # 640 edge-token capacity per window
NT = NW * TPW                # 1040 token tiles per core
CAPT = NT * 128              # 133120 token slots per core
EW = D + 1                   # payload: 64 feats + 1.0 count flag
EWP = 66                     # padded to even (fp32r matmul dst restriction)
F32 = mybir.dt.float32
F32R = mybir.dt.float32r
BF16 = mybir.dt.bfloat16
EQ = mybir.AluOpType.is_equal
MUL = mybir.AluOpType.mult
Copy = mybir.ActivationFunctionType.Copy
Relu = mybir.ActivationFunctionType.Relu

_PROG = None


def _bcast(ap, dim, n):
    """Insert a zero-stride dim of size n at free-dim position dim."""
    layout = list(ap.ap)
    layout.insert(1 + dim, [0, n])
    return AP(ap.tensor, ap.offset, layout)


def _build_program(reps=1):
    nc = bacc.Bacc(None, target_bir_lowering=False, debug=True)

    edges_d = nc.dram_tensor("edges_tok", [128, NT, EWP], BF16, kind="ExternalInput")
    ridx_d = nc.dram_tensor("ridx", [128, NT], BF16, kind="ExternalInput")
    nodeT_d = nc.dram_tensor("nodeT", [D, NSLOT], BF16, kind="ExternalInput")
    onehot_d = nc.dram_tensor("onehot", [NB, NSLOT], BF16, kind="ExternalInput")
    gaT_d = nc.dram_tensor("gaT", [D, NB], F32R, kind="ExternalInput")
    w1n_d = nc.dram_tensor("w1n", [D, LATENT], BF16, kind="ExternalInput")
    w1a_d = nc.dram_tensor("w1a", [D, LATENT], BF16, kind="ExternalInput")
    w1g_d = nc.dram_tensor("w1g", [D, LATENT], F32R, kind="ExternalInput")
    w2_d = nc.dram_tensor("w2", [LATENT, OUT_DIM], F32R, kind="ExternalInput")
    b1_d = nc.dram_tensor("b1c", [LATENT, 1], F32, kind="ExternalInput")
    b2_d = nc.dram_tensor("b2b", [128, OUT_DIM], F32, kind="ExternalInput")
    ident_d = nc.dram_tensor("ident", [128, 128], F32, kind="ExternalInput")
    iota_d = nc.dram_tensor("iota", [128, WIN], BF16, kind="ExternalInput")
    out_d = nc.dram_tensor("out", [NSLOT, OUT_DIM], F32, kind="ExternalOutput")

    with tile.TileContext(nc) as tc:
     # body repeated `reps` times for delta-timing (overhead cancels)
     for _rep in range(reps):
      with ExitStack() as stk:
        persist = stk.enter_context(tc.tile_pool(name="persist", bufs=1))
        gaT = persist.tile([D, NB], F32R)
        w1n = persist.tile([D, LATENT], BF16)
        w1a = persist.tile([D, LATENT], BF16)
        w1g = persist.tile([D, LATENT], F32R)
        w2 = persist.tile([LATENT, OUT_DIM], F32R)
        b1c = persist.tile([LATENT, 1], F32)
        b2b = persist.tile([128, OUT_DIM], F32)
        ident = persist.tile([128, 128], F32)
        iota = persist.tile([128, WIN], BF16)
        g_sb = persist.tile([NB, LATENT], BF16)
        ridx_sb = persist.tile([128, NT], BF16)

        for sb, dr in ((gaT, gaT_d), (w1n, w1n_d), (w1a, w1a_d), (w1g, w1g_d),
                       (w2, w2_d), (b1c, b1_d), (b2b, b2_d), (ident, ident_d),
                       (iota, iota_d), (ridx_sb, ridx_d)):
            nc.sync.dma_start(sb[:], dr[:])

        # G = global_attr @ W1g  (per-batch hidden contribution)
        with tc.tile_pool(name="psg", bufs=1, space="PSUM") as psg:
            ps_g = psg.tile([NB, LATENT], F32)
            nc.tensor.matmul(ps_g[:], gaT[:], w1g[:], start=True, stop=True)
            nc.scalar.activation(g_sb[:], ps_g[:], Copy)

        ppool = stk.enter_context(tc.tile_pool(name="pt", bufs=2))
        ohpool = stk.enter_context(tc.tile_pool(name="ohb", bufs=2))
        npool = stk.enter_context(tc.tile_pool(name="ndT", bufs=2))
        gpool = stk.enter_context(tc.tile_pool(name="ghT", bufs=2))
        pkpool = stk.enter_context(tc.tile_pool(name="pk", bufs=2))
        scpool = stk.enter_context(tc.tile_pool(name="sc", bufs=2))
        aggp = stk.enter_context(tc.tile_pool(name="agg", bufs=2))
        hp = stk.enter_context(tc.tile_pool(name="hp", bufs=2))
        opool = stk.enter_context(tc.tile_pool(name="op", bufs=4))
        psa = stk.enter_context(tc.tile_pool(name="psa", bufs=2, space="PSUM"))
        pst = stk.enter_context(tc.tile_pool(name="pst", bufs=2, space="PSUM"))
        ps1p = stk.enter_context(tc.tile_pool(name="ps1", bufs=2, space="PSUM"))
        ps2p = stk.enter_context(tc.tile_pool(name="ps2", bufs=2, space="PSUM"))

        WT = 8 * TPW  # 40 token tiles per supertile
        for s in range(NSUP):
            nodeT = npool.tile([D, 512], BF16, name="ndT")
            ohg = gpool.tile([NB, 512], BF16, name="ghT")
            nc.sync.dma_start(nodeT[:], nodeT_d[:, 512 * s:512 * (s + 1)])
            nc.sync.dma_start(ohg[:], onehot_d[:, 512 * s:512 * (s + 1)])

            pt = ppool.tile([128, WT, EWP], BF16, name="pt")
            nc.sync.dma_start(pt[:], edges_d[:, WT * s:WT * (s + 1), :])

            # one-hot routing for all 40 tiles at once
            ohb = ohpool.tile([128, WT, WIN], BF16, name="ohb")
            nc.vector.tensor_tensor(
                ohb[:], _bcast(iota[:], 0, WT),
                _bcast(ridx_sb[:, WT * s:WT * (s + 1)], 1, WIN), op=EQ)

            packed = pkpool.tile([WIN, 8, EWP], F32, name="pk")
            for wi in range(8):
                ps_agg = psa.tile([WIN, EWP], F32, name="ps_agg")
                for j in range(TPW):
                    t = TPW * wi + j
                    nc.tensor.matmul(
                        ps_agg[:], ohb[:, t, :], pt[:, t, :],
                        start=(j == 0), stop=(j == TPW - 1))
                nc.scalar.activation(packed[:, wi, :], ps_agg[:], Copy)

            cnt = scpool.tile([WIN, 8], F32, name="cnt")
            nc.vector.tensor_scalar_max(cnt[:], packed[:, :, D], 1.0)
            recip = scpool.tile([WIN, 8], F32, name="recip")
            nc.vector.reciprocal(recip[:], cnt[:])
            scaled = scpool.tile([WIN, 8, D], F32, name="scaled")
            nc.vector.tensor_tensor(scaled[:], packed[:, :, 0:D],
                                    _bcast(recip[:], 1, D), op=MUL)

            aggT = aggp.tile([D, 512], BF16, name="aggT")
            for wi in range(8):
                ps_t = pst.tile([D, WIN], F32, name="ps_t")
                nc.tensor.transpose(ps_t[:], scaled[:, wi, :],
                                    ident[0:WIN, 0:WIN])
                nc.scalar.activation(aggT[:, WIN * wi:WIN * (wi + 1)],
                                     ps_t[:], Copy)

            ps1 = ps1p.tile([LATENT, 512], F32, name="ps1")
            nc.tensor.matmul(ps1[:], w1n[:], nodeT[:], start=True, stop=False)
            nc.tensor.matmul(ps1[:], w1a[:], aggT[:], start=False, stop=False)
            nc.tensor.matmul(ps1[:], g_sb[:], ohg[:], start=False, stop=True)
            h = hp.tile([LATENT, 512], F32R, name="h")
            nc.scalar.activation(h[:], ps1[:], Relu, bias=b1c[:])
            for q in range(4):
                j = 4 * s + q
                ps2 = ps2p.tile([128, OUT_DIM], F32, name="ps2")
                nc.tensor.matmul(ps2[:], h[:, 128 * q:128 * (q + 1)], w2[:],
                                 start=True, stop=True)
                ob = opool.tile([128, OUT_DIM], F32, name="ob")
                nc.vector.tensor_add(ob[:], ps2[:], b2b[:])
                nc.sync.dma_start(out_d[128 * j:128 * (j + 1), :], ob[:])

    nc.compile()
    return nc


def _pack_windows(deg):
    """LPT bin-packing: assign each node to a window, balancing edge
    load with caps of WIN nodes / WTOK edges per window."""
    win_of = np.empty(NPC, np.int32)
    slot_of = np.empty(NPC, np.int32)
    counts = np.zeros(NW, np.int32)
    loads = np.zeros(NW, np.int64)
    heap = [(0, w) for w in range(NW)]
    for n in np.argsort(-deg, kind="stable"):
        while True:
            load, w = heapq.heappop(heap)
            if counts[w] < WIN:
                break
        win_of[n] = w
        slot_of[n] = counts[w]
        counts[w] += 1
        loads[w] += deg[n]
        assert loads[w] <= WTOK, f"window {w} overflow: {loads[w]}"
        if counts[w] < WIN:
            heapq.heappush(heap, (int(loads[w]), w))
    return win_of, slot_of


def _prep_inputs(node_attr, edge_attr, global_attr, W1, b1, W2, b2,
                 receivers_idx, ng_index):
    node_attr = np.asarray(node_attr, np.float32)
    edge_attr = np.asarray(edge_attr, np.float32)
    global_attr = np.asarray(global_attr, np.float32)
    W1 = np.asarray(W1, np.float32)
    b1 = np.asarray(b1, np.float32)
    W2 = np.asarray(W2, np.float32)
    b2 = np.asarray(b2, np.float32)
    receivers_idx = np.asarray(receivers_idx, np.int64)
    ng_index = np.asarray(ng_index, np.int64)

    BF = ml_dtypes.bfloat16
    shared = {
        "gaT": np.ascontiguousarray(global_attr.T),
        "w1n": np.ascontiguousarray(W1[0:D]).astype(BF),
        "w1a": np.ascontiguousarray(W1[D:2 * D]).astype(BF),
        "w1g": np.ascontiguousarray(W1[2 * D:3 * D]),
        "w2": np.ascontiguousarray(W2),
        "b1c": np.ascontiguousarray(b1.reshape(LATENT, 1)),
        "b2b": np.ascontiguousarray(np.broadcast_to(b2, (128, OUT_DIM))),
        "ident": np.eye(128, dtype=np.float32),
        "iota": np.tile(np.arange(WIN, dtype=BF), (128, 1)),
    }

    order = np.argsort(receivers_idx, kind="stable")
    sorted_recv = receivers_idx[order]
    bounds = np.searchsorted(sorted_recv, np.arange(0, N_NODES + 1, NPC))

    in_maps = []
    perms = []
    for k in range(NCORES):
        sel = order[bounds[k]:bounds[k + 1]]
        lrecv = (sorted_recv[bounds[k]:bounds[k + 1]] - k * NPC).astype(np.int64)
        e = sel.size
        deg = np.bincount(lrecv, minlength=NPC)
        win_of, slot_of = _pack_windows(deg)

        ew = win_of[lrecv].astype(np.int64)
        ord2 = np.argsort(ew, kind="stable")
        sel2 = sel[ord2]
        lrecv2 = lrecv[ord2]
        ew2 = ew[ord2]
        starts = np.searchsorted(ew2, np.arange(NW))
        pos = np.arange(e) - starts[ew2]
        assert e == 0 or pos.max() < WTOK
        tokslot = ew2 * WTOK + pos

        tok = np.zeros((CAPT, EWP), BF)
        tok[tokslot, :D] = edge_attr[sel2].astype(BF)
        tok[tokslot, D] = 1.0
        edges_tok = np.ascontiguousarray(
            tok.reshape(NT, 128, EWP).transpose(1, 0, 2))
        rx = np.full(CAPT, -1.0, BF)
        rx[tokslot] = slot_of[lrecv2].astype(BF)
        ridx = np.ascontiguousarray(rx.reshape(NT, 128).T)

        perm = np.full(NSLOT, -1, np.int64)
        perm[win_of.astype(np.int64) * WIN + slot_of] = np.arange(NPC)
        valid = np.flatnonzero(perm >= 0)
        gids = k * NPC + perm[valid]
        nodeT = np.zeros((D, NSLOT), BF)
        nodeT[:, valid] = node_attr[gids].T.astype(BF)
        oh = np.zeros((NB, NSLOT), BF)
        oh[ng_index[gids], valid] = 1.0

        m = {"edges_tok": edges_tok, "ridx": ridx, "nodeT": nodeT, "onehot": oh}
        m.update(shared)
        in_maps.append(m)
        perms.append(perm)
    return in_maps, perms


def _gather(outs, perms):
    full = np.zeros((N_NODES, OUT_DIM), np.float32)
    for k in range(NCORES):
        perm = perms[k]
        valid = np.flatnonzero(perm >= 0)
        full[k * NPC + perm[valid]] = np.asarray(outs[k])[valid]
    return full


def kernel(**inputs):
    global _PROG
    if _PROG is None:
        _PROG = _build_program()
    in_maps, perms = _prep_inputs(**inputs)
    res = run_bass_kernel_spmd(_PROG, in_maps, list(range(NCORES)), trace=False)
    return _gather([res.results[k]["out"] for k in range(NCORES)], perms)
